# revision 34
# baseline (speedup 1.0000x reference)
"""Trainium2 Bass kernel for nn_CrossTransformer_36756330119370.

The reference module's attention runs over a single key/value position
(k/v are projections of y reshaped to [B*T, 1, C]), so entmax15 over an
axis of length 1 is identically 1.0 and the q/k projections cancel out
of the forward entirely. The computation reduces exactly (verified
bit-identical on CPU) to:

    w[b, t, :] = Wvo @ y[b, :, t] + bvo        # [C] per (b,t)
    z[b, c, t, v] = x[b, c, t, v] + w[b, t, c]

where Wvo = Wo @ Wv and bvo = Wo @ bv + bo are folded on the host
(standard fusion of two chained linear layers; weights are kernel
constants).

Sharding: data-parallel over B across the 8 NeuronCores (8 batches per
core), folded weights replicated. Per core: one small fp32 matmul on
the PE engine produces w for the core's 960 (b,t) columns; then the
24.6MB x-shard is streamed HBM->SBUF in 3MB batch tiles, w is added
broadcast over the V axis with a stride-0 access pattern on the vector
engine, and the result streamed back. The kernel is DMA-fabric-bound
(~425 GB/s effective SBUF AXI rate per core).

Design notes from profiling:
- Queue split: x loads (and the packed-constant load) issue on the SP
  HWDGE ring; z stores issue on the SWDGE (gpsimd) ring. The SDMA
  engines round-robin between the two queues at packet granularity, so
  a store waiting on its DVE add never head-of-line-blocks later
  loads. This also removed an intermittent straggler on SDMA engine 15
  that a single-queue build suffered from.
- DMA count is kept minimal (18 bulk transfers): every additional
  dma_start costs ~0.5-0.7us of per-engine time regardless of size
  (64-packet minimum), so fine-grained tiling is a net loss. Only the
  final tile's store is split in half to shorten the exposed
  last-add+last-store tail.
- Raw bass (no Tile entry/exit machinery) with few semaphores: the
  kernel-tail semaphore-clear DMA latency scales with the semaphore
  range.
"""

import os
import sys

for _p in ("/opt/trn_rl_repo", "/root/.axon_site/_ro/trn_rl_repo"):
    if os.path.isdir(_p) and _p not in sys.path:
        sys.path.append(_p)

import numpy as np

import concourse.bass as bass
import concourse.mybir as mybir
from concourse.bass_utils import run_bass_kernel_spmd

N_CORES = 8
B, C, T, V = 64, 256, 120, 25
BPC = B // N_CORES          # batches per core
P = 128                     # SBUF partitions
NCC = C // P                # channel chunks (2)
BT = BPC * T                # (b, t) columns per core (960)
NT = 480                    # matmul moving-operand tile (<=512 for fp32)
TV = T * V                  # contiguous elements per (b, c) row (3000)
NBUF = 7                    # x-tile double-buffer slots

# column offsets inside the packed constant tensor
OFF_W = 0                   # [kc, m] -> kc*C + m          (512 cols)
OFF_B = NCC * C             # 512: [mc]                    (2 cols)
OFF_Y = OFF_B + NCC         # 514: [kc, b, t] -> kc*BT + b*T + t (1920 cols)
PACK_COLS = OFF_Y + NCC * BT  # 2434

FP32 = mybir.dt.float32

# Stash of the last hardware run results (exec_time_ns etc.) for test.py.
LAST_RESULTS = None


def legalize_waits(nc: bass.Bass, max_waits: int = 1) -> None:
    """Split multi-semaphore waits into standalone NoOp wait carriers.

    The walrus build here rejects any instruction carrying more than one
    sync-wait command ("Too many sync wait commands"). A NoOp on the
    same engine stalls the sequencer identically, so hoisting all but
    one wait onto NoOps preserves semantics.
    """
    k = 0
    for blk in nc.m.functions[0].blocks:
        insts = blk.instructions
        i = 0
        while i < len(insts):
            inst = insts[i]
            si = getattr(inst, "sync_info", None)
            if si is not None and si.on_wait and len(si.on_wait) > max_waits:
                waits = list(si.on_wait)
                for w in waits[:-max_waits]:
                    nop = mybir.InstNoOp(name=f"NW-{k}")
                    k += 1
                    nop.engine = inst.engine
                    nop.sync_info = mybir.SyncInfo(on_wait=[w], on_update=[])
                    insts.insert(i, nop)
                    i += 1
                inst.sync_info = mybir.SyncInfo(
                    on_wait=waits[-max_waits:], on_update=si.on_update)
            i += 1


def build_nc_raw() -> bass.Bass:
    """Hand-synchronized raw-bass build.

    Per-slot cumulative counting semaphores: slot s's DMAs
    (load_s -> store_s -> load_{s+7} -> store_{s+7}) are strictly
    serialized by the compute chain, so cumulative thresholds are
    alias-free even with loads and stores on different queues. Every
    instruction carries at most one sync wait (walrus limit).
    """
    nc = bass.Bass("TRN2", debug=False, num_devices=N_CORES)

    x = nc.dram_tensor("x", [BPC, C, T, V], FP32, kind="ExternalInput").ap()
    cpak = nc.dram_tensor("cpak", [P, PACK_COLS], FP32, kind="ExternalInput").ap()
    z = nc.dram_tensor("z", [BPC, C, T, V], FP32, kind="ExternalOutput").ap()

    cs = nc.alloc_sbuf_tensor("cs", [P, PACK_COLS], FP32).ap()
    w_sb = nc.alloc_sbuf_tensor("w_sb", [P, NCC, BT], FP32).ap()
    xts = [nc.alloc_sbuf_tensor(f"xt{i}", [P, NCC, TV], FP32).ap()
           for i in range(NBUF)]
    ps = [nc.alloc_psum_tensor(f"ps{g}", [P, NT], FP32).ap() for g in range(4)]

    sCP = nc.alloc_semaphore("sCP")
    sSL = [nc.alloc_semaphore(f"sSL{i}") for i in range(NBUF)]
    sPE = nc.alloc_semaphore("sPE")
    sACT = nc.alloc_semaphore("sACT")
    sDVE = nc.alloc_semaphore("sDVE")

    LAST = BPC - 1

    # ---- SP (sync) stream: constant load + all x loads ----
    # The final tile (b=7, slot 0) is loaded as two cc-halves so its
    # adds/stores can start before the whole tile lands; the second
    # half's completion counts on slot 1's semaphore to keep every
    # threshold equal to the total incs of a completed set (alias-free:
    # no one waits on slot 1's intermediate values).
    # Partition p holds the ADJACENT channel pair (2p, 2p+1): one
    # contiguous 24KB DRAM run per partition -> 1 descriptor instead of
    # 2, halving the HWDGE descriptor-emission ramp at stream start.
    # Host-side, Wvo's rows are permuted (even channels -> mc=0 block,
    # odd -> mc=1) so w_sb[p, cc, :] is the w row of channel 2p+cc and
    # the DVE broadcast APs are unchanged.
    def tile_dram(dram, b):
        return dram[b].rearrange("(p cc) t v -> p cc (t v)", cc=NCC)

    sync = nc.sync
    sync.dma_start(cs, cpak).then_inc(sCP, 16)
    for b in range(BPC):
        s = b % NBUF
        if b >= NBUF:
            # slot reuse: wait for store_{b-NBUF} to fully drain
            sync.wait_ge(sSL[s], 32)
        if b < LAST:
            sync.dma_start(xts[s], tile_dram(x, b)).then_inc(sSL[s], 16)
        else:
            for h in range(NCC):
                sync.dma_start(
                    xts[s][:, h, :], tile_dram(x, b)[:, h, :],
                ).then_inc(sSL[s + h], 16)

    # ---- PE stream: one folded projection, 4 psum groups ----
    # group order (nch, mc): batches 0-3 live in nch0 columns, 4-7 in
    # nch1; the DVE gates align with this order.
    PE_ORDER = [(0, 0), (0, 1), (1, 0), (1, 1)]  # (nch, mc)
    nc.tensor.wait_ge(sCP, 16)
    for g, (nch, mc) in enumerate(PE_ORDER):
        for kc in range(NCC):
            col = OFF_W + kc * C + mc * P
            mm = nc.tensor.matmul(
                ps[g],
                lhsT=cs[:, col:col + P],
                rhs=cs[:, OFF_Y + kc * BT + nch * NT:
                       OFF_Y + kc * BT + (nch + 1) * NT],
                start=(kc == 0), stop=(kc == NCC - 1),
            )
        mm.then_inc(sPE)

    # ---- ACT stream: PSUM->SBUF with per-partition bias ----
    for g, (nch, mc) in enumerate(PE_ORDER):
        nc.scalar.wait_ge(sPE, g + 1)
        nc.scalar.add(
            w_sb[:, mc, nch * NT:(nch + 1) * NT],
            ps[g],
            cs[:, OFF_B + mc:OFF_B + mc + 1],
        ).then_inc(sACT)

    # ---- DVE stream: broadcast adds (last tile split in t-quarters) ----
    TQ = T // 2  # last-tile t-split

    def add_w(b, h0, h1, t0=0, tn=T):
        w_bc = (
            w_sb[:, h0:h1, b * T + t0:b * T + t0 + tn]
            .unsqueeze(3)
            .broadcast_to([P, h1 - h0, tn, V])
        )
        xv = xts[b % NBUF].rearrange("p cc (t v) -> p cc t v", v=V)[
            :, h0:h1, t0:t0 + tn, :]
        return nc.vector.tensor_tensor(xv, xv, w_bc, mybir.AluOpType.add)

    for b in range(BPC):
        s = b % NBUF
        nc.vector.wait_ge(sACT, 2 if b < 4 else 4)
        if b < LAST:
            nc.vector.wait_ge(sSL[s], 16 if b < NBUF else 48)
            add_w(b, 0, NCC).then_inc(sDVE)
        else:
            # four quarter-adds, each unblocking its quarter-store
            for h in range(NCC):
                nc.vector.wait_ge(sSL[s + h], 48)
                for t0 in (0, TQ):
                    add_w(b, h, h + 1, t0, TQ).then_inc(sDVE)

    # ---- GPSIMD (Pool/SWDGE) stream: all z stores ----
    for b in range(BPC):
        s = b % NBUF
        if b < LAST:
            nc.gpsimd.wait_ge(sDVE, b + 1)
            nc.gpsimd.dma_start(
                tile_dram(z, b), xts[s]).then_inc(sSL[s], 16)
        else:
            # final tile: store each quarter as soon as its add lands
            for q, (h, t0) in enumerate(
                    (h, t0) for h in range(NCC) for t0 in (0, TQ)):
                nc.gpsimd.wait_ge(sDVE, LAST + 1 + q)
                nc.gpsimd.dma_start(
                    tile_dram(z, b)[:, h, t0 * V:(t0 + TQ) * V],
                    xts[s][:, h, t0 * V:(t0 + TQ) * V],
                ).then_inc(sSL[s + h], 16)
    # drain: all stores complete before kernel end
    for s in range(NBUF):
        nc.gpsimd.wait_ge(sSL[s], 80 if s < NCC else 32)
    nc.gpsimd.wait_ge(sCP, 16)

    nc.all_engine_barrier()
    nc.clear_and_free_semaphores([sCP] + sSL + [sPE, sACT, sDVE])

    # Drop Bass's const-AP pool init memsets: this kernel never uses
    # const APs (all biases are real SBUF tensors, scalars are
    # immediates), so the four preamble memsets are dead code.
    for blk in nc.m.functions[0].blocks:
        blk.instructions[:] = [
            i for i in blk.instructions
            if not (type(i).__name__ == "InstMemset"
                    and "const-" in str(i.outs[0]))
        ]

    legalize_waits(nc)
    return nc


def pack_consts(y_shard, Wvo, bvo):
    """Build the [P, PACK_COLS] constant tensor for one core."""
    cpak = np.empty((P, PACK_COLS), np.float32)
    # wt[c_in, c_out] = Wvo[c_out, c_in]; cs[p, kc*C + m] = wt[kc*P+p, m]
    cpak[:, OFF_W:OFF_W + NCC * C] = (
        Wvo.T.reshape(NCC, P, C).transpose(1, 0, 2).reshape(P, NCC * C))
    cpak[:, OFF_B:OFF_B + NCC] = bvo.reshape(NCC, P).T
    # y_sb[p, kc*BT + b*T + t] = y[b, kc*P+p, t]
    cpak[:, OFF_Y:] = (
        y_shard.reshape(BPC, NCC, P, T).transpose(2, 1, 0, 3).reshape(P, NCC * BT))
    return cpak


_NC_CACHE = None


def _get_nc():
    global _NC_CACHE
    if _NC_CACHE is None:
        _NC_CACHE = build_nc_raw()
    return _NC_CACHE


def kernel(x, y, Wq=None, bq=None, Wk=None, bk=None, Wv=None, bv=None,
           Wo=None, bo=None, **_unused):
    global LAST_RESULTS
    x = np.ascontiguousarray(np.asarray(x, dtype=np.float32))
    y = np.asarray(y, dtype=np.float32)
    Wv = np.asarray(Wv, dtype=np.float32)
    bv = np.asarray(bv, dtype=np.float32)
    Wo = np.asarray(Wo, dtype=np.float32)
    bo = np.asarray(bo, dtype=np.float32)
    # fold the two chained linear layers into one, and permute output
    # channels so partition p produces the adjacent pair (2p, 2p+1):
    # even channels fill the mc=0 block, odd channels the mc=1 block
    Wvo = (Wo @ Wv).astype(np.float32)
    bvo = (Wo @ bv + bo).astype(np.float32)
    perm = np.concatenate([np.arange(0, C, 2), np.arange(1, C, 2)])
    Wvo = Wvo[perm]
    bvo = bvo[perm]

    nc = _get_nc()
    in_maps = []
    for c in range(N_CORES):
        sl = slice(c * BPC, (c + 1) * BPC)
        in_maps.append({
            "x": x[sl],
            "cpak": pack_consts(y[sl], Wvo, bvo),
        })

    res = run_bass_kernel_spmd(
        nc, in_maps, list(range(N_CORES)),
        trace=bool(os.environ.get("KERNEL_PROFILE")),
    )
    LAST_RESULTS = res
    return np.concatenate([res.results[c]["z"] for c in range(N_CORES)], axis=0)
